# revision 11
# baseline (speedup 1.0000x reference)
"""MoE layer (shared expert + 8 routed experts, top-2 sigmoid router) on 8
Trainium2 NeuronCores.

Strategy: expert-parallel sparse dispatch (two launches).

  Launch A (router only, data-parallel, 1024 tokens/core): fp32 PE logits.
  fp32 is selection-critical: the smallest top-2/3 score margin here is
  ~6e-6, far below fp16/f32r matmul error. The tiny router weight tile is
  the stationary operand (8-row loads) and x streams as the moving operand
  in 512-token chunks, k-outer, so compute tracks the x DMA.

  Host dispatch: fp64 sigmoid of the device logits, top-2 selection with
  lax.top_k tie-breaking (stable argsort), gate normalization. Tokens are
  gathered per expert and pre-scaled by sqrt(gate): since sqrt(c) >= 0,
  relu(w1.T @ (x*sqrt(c))) = sqrt(c)*relu(w1.T @ x), so the squared-relu MLP
  of the scaled token yields exactly gate * expert(x), no on-device scaling.

  Launch B (expert-parallel): core e runs the shared expert over its own
  1024 tokens plus expert e over its ~2k gathered tokens, all in fp16
  (same PE rate as f32r, half the DMA). Every input arrives via a single
  large DMA from a host-prearranged [128, k, m] layout. Layer order
  sL1 -> rL1 -> sL2 -> rL2 keeps PSUM drains off the critical path; the
  first sL1 token chunk runs k-outer across 6 PSUM accumulators so the PE
  starts as soon as the first x/w k-tile lands. L1 drains as
  relu (scalar engine) then square (DVE fp16 2x mode); L2 PSUM->SBUF copies
  alternate between DVE and scalar into per-out-tile staging, written back
  with one DMA per 128-row out tile. The host scatter-adds the two routed
  contributions per token onto the shared output.

This does 3 MLPs/token (shared + top-2) instead of the dense baseline's 9.
"""
import sys

sys.path.insert(0, '/opt/trn_rl_repo')

import numpy as np

import concourse.bass as bass
import concourse.mybir as mybir
import concourse.tile as tile
from concourse import bacc
from concourse.bass_utils import run_bass_kernel_spmd

f32 = mybir.dt.float32
f16 = mybir.dt.float16
AF = mybir.ActivationFunctionType
ALU = mybir.AluOpType

N_CORES = 8
B, T, C = 4, 2048, 768
E, K = 8, 2
N_TOK = B * T
TLOC = N_TOK // N_CORES       # tokens per core in launch A (1024)
KT = C // 128                 # 6 contraction tiles
TB = TLOC // 128              # 8 router token blocks

TRACE = False                 # test.py sets this for profiled runs


def _chunks(t):
    out = []
    off = 0
    while off < t:
        c = min(512, t - off)
        out.append((off, c))
        off += c
    return out


def _emit_layer1(nc, pspool, tpool, wsb, xh, hsq, t_tokens, ramp=False):
    # hsq[ho] = relu(w1[:, ho].T @ x)^2 in fp16. Relu on the scalar engine
    # (the one PSUM read), square on DVE as fp16 SBUF*SBUF (2x fast mode).
    chunks = _chunks(t_tokens)
    start = 0
    if ramp:
        # First chunk k-outer across 6 PSUM accumulators: the PE can start
        # on k-tile 0 as soon as ~0.5MB of input has landed instead of
        # waiting for the full x/w tiles.
        off, chn = chunks[0]
        start = 1
        ps6 = [pspool.tile([128, chn], f32, tag=f"p{j % 2}", name=f"p{j % 2}")
               for j in range(KT)]
        for k in range(KT):
            for ho in range(KT):
                nc.tensor.matmul(ps6[ho][:],
                                 wsb[:, k, ho * 128:(ho + 1) * 128],
                                 xh[:, k, off:off + chn],
                                 start=(k == 0), stop=(k == KT - 1))
        for ho in range(KT):
            t_ = tpool.tile([128, chn], f16, tag=f"t{ho % 2}",
                            name=f"t{ho % 2}")
            nc.scalar.activation(t_[:], ps6[ho][:], AF.Relu)
            nc.vector.tensor_tensor(hsq[:, ho, off:off + chn],
                                    t_[:], t_[:], ALU.mult)
    for ho in range(KT):
        mo = slice(ho * 128, (ho + 1) * 128)
        for ci, (off, chn) in enumerate(chunks[start:]):
            ps = pspool.tile([128, chn], f32, tag=f"p{ci % 2}",
                             name=f"p{ci % 2}")
            for k in range(KT):
                nc.tensor.matmul(ps[:], wsb[:, k, mo],
                                 xh[:, k, off:off + chn],
                                 start=(k == 0), stop=(k == KT - 1))
            t_ = tpool.tile([128, chn], f16, tag=f"t{ci % 2}",
                            name=f"t{ci % 2}")
            nc.scalar.activation(t_[:], ps[:], AF.Relu)
            nc.vector.tensor_tensor(hsq[:, ho, off:off + chn],
                                    t_[:], t_[:], ALU.mult)


def _emit_layer2(nc, pspool, ypool, wsb, hsq, out_dram, t_tokens):
    # out[co] = w2[:, co].T @ hsq in fp16. co-outer: each out tile stages
    # into one SBUF row-tile (PSUM->SBUF copies alternating DVE/scalar) and
    # leaves with a single DMA.
    chunks = _chunks(t_tokens)
    for co in range(KT):
        mo = slice(co * 128, (co + 1) * 128)
        yst = ypool.tile([128, t_tokens], f16, tag="yst", name="yst")
        for ci, (off, chn) in enumerate(chunks):
            ps = pspool.tile([128, chn], f32, tag=f"p{ci % 2}",
                             name=f"p{ci % 2}")
            for k in range(KT):
                nc.tensor.matmul(ps[:], wsb[:, k, mo],
                                 hsq[:, k, off:off + chn],
                                 start=(k == 0), stop=(k == KT - 1))
            if ci % 2 == 0:
                nc.vector.tensor_copy(yst[:, off:off + chn], ps[:])
            else:
                nc.scalar.activation(yst[:, off:off + chn], ps[:], AF.Copy)
        nc.sync.dma_start(out_dram[mo, :], yst[:])


def _build_a():
    nc = bacc.Bacc("TRN2", target_bir_lowering=False, debug=False,
                   num_devices=N_CORES)

    x_T = nc.declare_dram_parameter("x_T", [128, KT, TLOC], f32,
                                    isOutput=False)
    rwT = nc.declare_dram_parameter("rwT", [C, E], f32, isOutput=False)
    o_lg = nc.declare_dram_parameter("o_lg", [E, TLOC], f32, isOutput=True)

    with tile.TileContext(nc) as tc:
        with (
            tc.tile_pool(name="const", bufs=1) as cpool,
            tc.tile_pool(name="acts", bufs=1) as apool,
            tc.tile_pool(name="psl", bufs=1, space="PSUM") as plpool,
        ):
            rwt = cpool.tile([128, KT, E], f32)
            nc.sync.dma_start(rwt[:], rwT.rearrange("(k p) e -> p k e", p=128))
            xt = apool.tile([128, KT, TLOC], f32, tag="xt")
            nc.sync.dma_start(xt[:], x_T[:])

            # k-outer, rwt stationary (8-row loads), x moving: 12 large
            # fp32 matmuls that track the x DMA stream.
            psl = [plpool.tile([E, 512], f32, tag=f"pl{h}", name=f"pl{h}")
                   for h in range(2)]
            for k in range(KT):
                for h in range(2):
                    nc.tensor.matmul(psl[h][:], rwt[:, k, :],
                                     xt[:, k, h * 512:(h + 1) * 512],
                                     start=(k == 0), stop=(k == KT - 1))
            lg = apool.tile([E, TLOC], f32, tag="lg")
            for h in range(2):
                nc.scalar.activation(lg[:, h * 512:(h + 1) * 512],
                                     psl[h][:], AF.Copy)
            nc.sync.dma_start(o_lg[:], lg[:])
    nc.compile()
    return nc


def _build_b(trp):
    nc = bacc.Bacc("TRN2", target_bir_lowering=False, debug=False,
                   num_devices=N_CORES)

    x_h = nc.declare_dram_parameter("x_h", [128, KT, TLOC], f16,
                                    isOutput=False)
    wfc = nc.declare_dram_parameter("wfc", [128, KT, C], f16, isOutput=False)
    wproj = nc.declare_dram_parameter("wproj", [128, KT, C], f16,
                                      isOutput=False)
    xg = nc.declare_dram_parameter("xg", [128, KT, trp], f16, isOutput=False)
    w1 = nc.declare_dram_parameter("w1", [128, KT, C], f16, isOutput=False)
    w2 = nc.declare_dram_parameter("w2", [128, KT, C], f16, isOutput=False)
    o_ysh = nc.declare_dram_parameter("o_ysh", [C, TLOC], f16, isOutput=True)
    o_yr = nc.declare_dram_parameter("o_yr", [C, trp], f16, isOutput=True)

    with tile.TileContext(nc) as tc:
        with (
            tc.tile_pool(name="acts", bufs=1) as apool,
            tc.tile_pool(name="tbuf", bufs=2) as tpool,
            tc.tile_pool(name="ybuf", bufs=2) as ypool,
            tc.tile_pool(name="ps", bufs=4, space="PSUM") as pspool,
        ):
            # One large DMA per input, issued in consumption order.
            xh = apool.tile([128, KT, TLOC], f16, tag="xh")
            nc.sync.dma_start(xh[:], x_h[:])
            wfcsb = apool.tile([128, KT, C], f16, tag="wfcsb")
            nc.sync.dma_start(wfcsb[:], wfc[:])
            w1sb = apool.tile([128, KT, C], f16, tag="w1sb")
            nc.sync.dma_start(w1sb[:], w1[:])
            xgt = apool.tile([128, KT, trp], f16, tag="xgt")
            nc.sync.dma_start(xgt[:], xg[:])
            wpsb = apool.tile([128, KT, C], f16, tag="wpsb")
            nc.sync.dma_start(wpsb[:], wproj[:])
            w2sb = apool.tile([128, KT, C], f16, tag="w2sb")
            nc.sync.dma_start(w2sb[:], w2[:])

            hsq_s = apool.tile([128, KT, TLOC], f16, tag="hsq_s")
            hsq_r = apool.tile([128, KT, trp], f16, tag="hsq_r")
            # sL1 -> rL1 -> sL2 -> rL2: each layer's PSUM drain finishes
            # well before its consumer starts, so the PE never waits.
            _emit_layer1(nc, pspool, tpool, wfcsb, xh, hsq_s, TLOC, ramp=True)
            _emit_layer1(nc, pspool, tpool, w1sb, xgt, hsq_r, trp)
            _emit_layer2(nc, pspool, ypool, wpsb, hsq_s, o_ysh, TLOC)
            _emit_layer2(nc, pspool, ypool, w2sb, hsq_r, o_yr, trp)
    nc.compile()
    return nc


_NC_A = None
_NC_B = {}


def _get_nc_a():
    global _NC_A
    if _NC_A is None:
        _NC_A = _build_a()
    return _NC_A


def _get_nc_b(trp):
    if trp not in _NC_B:
        _NC_B[trp] = _build_b(trp)
    return _NC_B[trp]


def _run(nc, in_maps, label):
    if TRACE:
        import tempfile
        td = tempfile.mkdtemp(prefix=f"moe_{label}_")
        res = run_bass_kernel_spmd(nc, in_maps, list(range(N_CORES)),
                                   trace=True, tmpdir=td)
        kernel._exec_ns[label] = res.exec_time_ns
        kernel._trace_dirs[label] = td
    else:
        res = run_bass_kernel_spmd(nc, in_maps, list(range(N_CORES)))
    return res


def _ptiles(a):
    """[C, t] -> [128, KT, t] partition-major layout, contiguous."""
    return np.ascontiguousarray(
        a.reshape(KT, 128, a.shape[1]).transpose(1, 0, 2))


def kernel(x, w_fc_sh, w_proj_sh, w1, w2, router_w, balance_bias):
    kernel._exec_ns = {}
    kernel._trace_dirs = {}

    xf = np.ascontiguousarray(np.asarray(x, np.float32).reshape(N_TOK, C))
    rwT = np.ascontiguousarray(np.asarray(router_w, np.float32).T)
    wfc16 = _ptiles(np.asarray(w_fc_sh, np.float32).astype(np.float16))
    wproj16 = _ptiles(np.asarray(w_proj_sh, np.float32).astype(np.float16))
    w1_16 = [_ptiles(np.asarray(w1[e], np.float32).astype(np.float16))
             for e in range(E)]
    w2_16 = [_ptiles(np.asarray(w2[e], np.float32).astype(np.float16))
             for e in range(E)]
    bias = np.asarray(balance_bias, np.float64)

    # ---- launch A: router logits, data-parallel ----
    nc_a = _get_nc_a()
    xTs = [np.ascontiguousarray(xf[i * TLOC:(i + 1) * TLOC].T)
           for i in range(N_CORES)]
    res_a = _run(nc_a, [{"x_T": _ptiles(xTs[i]), "rwT": rwT}
                        for i in range(N_CORES)], "a")

    lg = np.concatenate([res_a.results[i]["o_lg"].T
                         for i in range(N_CORES)], axis=0)      # [N, E] fp32

    # ---- host dispatch: top-2 selection + per-expert gather ----
    scores = 1.0 / (1.0 + np.exp(-lg.astype(np.float64)))
    idx = np.argsort(-(scores + bias[None, :]), axis=-1, kind="stable")[:, :K]
    tw = np.take_along_axis(scores, idx, -1)
    tw = tw / (tw.sum(-1, keepdims=True) + 1e-20)
    comb = np.zeros((N_TOK, E))
    np.put_along_axis(comb, idx, tw, -1)

    tok_lists = [np.nonzero(comb[:, e])[0] for e in range(E)]
    trp = max(512, -(-max(len(t) for t in tok_lists) // 128) * 128)

    nc_b = _get_nc_b(trp)
    in_maps = []
    for e in range(E):
        te = tok_lists[e]
        xe = xf[te] * np.sqrt(comb[te, e]).astype(np.float32)[:, None]
        xgT = np.zeros((C, trp), np.float32)
        xgT[:, :len(te)] = xe.T
        in_maps.append({"x_h": _ptiles(xTs[e]).astype(np.float16),
                        "wfc": wfc16, "wproj": wproj16,
                        "xg": _ptiles(xgT).astype(np.float16),
                        "w1": w1_16[e], "w2": w2_16[e]})

    # ---- launch B: shared expert (own tokens) + routed expert e ----
    res_b = _run(nc_b, in_maps, "b")

    y = np.concatenate([res_b.results[i]["o_ysh"].T
                        for i in range(N_CORES)], axis=0).astype(np.float32)
    for e in range(E):
        te = tok_lists[e]
        y[te] += res_b.results[e]["o_yr"][:, :len(te)].T.astype(np.float32)

    kernel._comb = comb
    return y.reshape(B, T, C).astype(np.float32)


# revision 12
# speedup vs baseline: 1.1853x; 1.1853x over previous
"""MoE layer (shared expert + 8 routed experts, top-2 sigmoid router) on 8
Trainium2 NeuronCores.

Strategy: expert-parallel sparse dispatch (two launches).

  Launch A (router only, data-parallel, 1024 tokens/core): fp32 PE logits.
  fp32 is selection-critical: the smallest top-2/3 score margin here is
  ~6e-6, far below fp16/f32r matmul error. The tiny router weight tile is
  the stationary operand (8-row loads) and x streams as the moving operand
  in 512-token chunks, k-outer with two interleaved PSUM accumulators, so
  compute tracks the per-k x DMA stream.

  Host dispatch: fp64 sigmoid of the device logits, top-2 selection with
  lax.top_k tie-breaking (stable argsort), gate normalization. Tokens are
  gathered per expert and pre-scaled by sqrt(gate): since sqrt(c) >= 0,
  relu(w1.T @ (x*sqrt(c))) = sqrt(c)*relu(w1.T @ x), so the squared-relu MLP
  of the scaled token yields exactly gate * expert(x), no on-device scaling.

  Launch B (expert-parallel): core e runs the shared expert over its own
  1024 tokens plus expert e over its ~2k gathered tokens, all in fp16
  (same PE rate as f32r, half the DMA). Inputs arrive via per-k DMAs whose
  triggers are split across the two HWDGE queues (sync + scalar) so issue
  latency never gates data delivery. Layer order sL1 -> rL1 -> sL2 -> rL2.
  Matmuls always interleave two PSUM accumulation streams (pairing chunks,
  crossing output-tile boundaries when odd) to hide the PSUM write-read
  turnaround; the first sL1 chunk runs k-outer across 6 accumulators so the
  PE starts as soon as the first x/w k-tile lands. L1 drains as relu
  (scalar engine) then square (DVE fp16 2x mode); L2 PSUM->SBUF copies
  alternate DVE/scalar into a per-out-tile staging row written back with
  one DMA per out tile. The host scatter-adds the two routed contributions
  per token onto the shared output.

This does 3 MLPs/token (shared + top-2) instead of the dense baseline's 9.
"""
import sys

sys.path.insert(0, '/opt/trn_rl_repo')

import numpy as np

import concourse.bass as bass
import concourse.mybir as mybir
import concourse.tile as tile
from concourse import bacc
from concourse.bass_utils import run_bass_kernel_spmd

f32 = mybir.dt.float32
f16 = mybir.dt.float16
AF = mybir.ActivationFunctionType
ALU = mybir.AluOpType

N_CORES = 8
B, T, C = 4, 2048, 768
E, K = 8, 2
N_TOK = B * T
TLOC = N_TOK // N_CORES       # tokens per core in launch A (1024)
KT = C // 128                 # 6 contraction tiles
TB = TLOC // 128              # 8 router token blocks

TRACE = False                 # test.py sets this for profiled runs


def _chunks(t, start=0):
    out = []
    off = start
    while off < t:
        c = min(512, t - off)
        out.append((off, c))
        off += c
    return out


def _emit_layer1(nc, pspool, tpool, wsb, xh, hsq, t_tokens, ramp=False):
    # hsq[ho] = relu(w1[:, ho].T @ x)^2 in fp16. Relu on the scalar engine
    # (the one PSUM read), square on DVE as fp16 SBUF*SBUF (2x fast mode).
    start = 0
    if ramp:
        # First chunk k-outer across 6 PSUM accumulators: the PE starts on
        # k-tile 0 as soon as the first per-k x/w DMAs land.
        chn = 512
        start = chn
        ps6 = [pspool.tile([128, chn], f32, tag=f"p{j % 2}", name=f"p{j % 2}")
               for j in range(KT)]
        for k in range(KT):
            for ho in range(KT):
                nc.tensor.matmul(ps6[ho][:],
                                 wsb[:, k, ho * 128:(ho + 1) * 128],
                                 xh[:, k, 0:chn],
                                 start=(k == 0), stop=(k == KT - 1))
        for ho in range(KT):
            t_ = tpool.tile([128, chn], f16, tag=f"t{ho % 2}",
                            name=f"t{ho % 2}")
            nc.scalar.activation(t_[:], ps6[ho][:], AF.Relu)
            nc.vector.tensor_tensor(hsq[:, ho, 0:chn], t_[:], t_[:], ALU.mult)

    units = [(ho, off, chn) for ho in range(KT)
             for off, chn in _chunks(t_tokens, start)]
    for i in range(0, len(units), 2):
        pair = units[i:i + 2]
        ps = [pspool.tile([128, chn], f32, tag=f"p{j}", name=f"p{j}")
              for j, (ho, off, chn) in enumerate(pair)]
        for k in range(KT):
            for j, (ho, off, chn) in enumerate(pair):
                nc.tensor.matmul(ps[j][:],
                                 wsb[:, k, ho * 128:(ho + 1) * 128],
                                 xh[:, k, off:off + chn],
                                 start=(k == 0), stop=(k == KT - 1))
        for j, (ho, off, chn) in enumerate(pair):
            t_ = tpool.tile([128, chn], f16, tag=f"t{j}", name=f"t{j}")
            nc.scalar.activation(t_[:], ps[j][:], AF.Relu)
            nc.vector.tensor_tensor(hsq[:, ho, off:off + chn],
                                    t_[:], t_[:], ALU.mult)


def _emit_layer2(nc, pspool, ypool, wsb, hsq, out_dram, t_tokens):
    # out[co] = w2[:, co].T @ hsq in fp16, staged per out tile in SBUF and
    # written back with one DMA per out tile. PSUM->SBUF copies alternate
    # DVE / scalar.
    chunks = _chunks(t_tokens)
    units = [(co, off, chn, ci == len(chunks) - 1)
             for co in range(KT) for ci, (off, chn) in enumerate(chunks)]
    yst = {}
    for i in range(0, len(units), 2):
        pair = units[i:i + 2]
        ps = [pspool.tile([128, chn], f32, tag=f"p{j}", name=f"p{j}")
              for j, (co, off, chn, last) in enumerate(pair)]
        for k in range(KT):
            for j, (co, off, chn, last) in enumerate(pair):
                nc.tensor.matmul(ps[j][:],
                                 wsb[:, k, co * 128:(co + 1) * 128],
                                 hsq[:, k, off:off + chn],
                                 start=(k == 0), stop=(k == KT - 1))
        for j, (co, off, chn, last) in enumerate(pair):
            if co not in yst:
                yst[co] = ypool.tile([128, t_tokens], f16, tag="yst",
                                     name="yst")
            if (i + j) % 2 == 0:
                nc.vector.tensor_copy(yst[co][:, off:off + chn], ps[j][:])
            else:
                nc.scalar.activation(yst[co][:, off:off + chn], ps[j][:],
                                     AF.Copy)
            if last:
                nc.sync.dma_start(
                    out_dram[co * 128:(co + 1) * 128, :], yst[co][:])


def _build_a():
    nc = bacc.Bacc("TRN2", target_bir_lowering=False, debug=False,
                   num_devices=N_CORES)

    x_T = nc.declare_dram_parameter("x_T", [128, KT, TLOC], f32,
                                    isOutput=False)
    rwT = nc.declare_dram_parameter("rwT", [C, E], f32, isOutput=False)
    o_lg = nc.declare_dram_parameter("o_lg", [E, TLOC], f32, isOutput=True)

    with tile.TileContext(nc) as tc:
        with (
            tc.tile_pool(name="const", bufs=1) as cpool,
            tc.tile_pool(name="acts", bufs=1) as apool,
            tc.tile_pool(name="psl", bufs=1, space="PSUM") as plpool,
        ):
            rwt = cpool.tile([128, KT, E], f32)
            nc.sync.dma_start(rwt[:], rwT.rearrange("(k p) e -> p k e", p=128))
            xt = apool.tile([128, KT, TLOC], f32, tag="xt")
            for k in range(KT):
                eng = nc.sync if k % 2 == 0 else nc.scalar
                eng.dma_start(xt[:, k, :], x_T[:, k, :])

            # k-outer, rwt stationary (8-row loads), x moving: 12 large
            # fp32 matmuls that track the per-k x DMA stream.
            psl = [plpool.tile([E, 512], f32, tag=f"pl{h}", name=f"pl{h}")
                   for h in range(2)]
            for k in range(KT):
                for h in range(2):
                    nc.tensor.matmul(psl[h][:], rwt[:, k, :],
                                     xt[:, k, h * 512:(h + 1) * 512],
                                     start=(k == 0), stop=(k == KT - 1))
            lg = apool.tile([E, TLOC], f32, tag="lg")
            for h in range(2):
                nc.scalar.activation(lg[:, h * 512:(h + 1) * 512],
                                     psl[h][:], AF.Copy)
            nc.sync.dma_start(o_lg[:], lg[:])
    nc.compile()
    return nc


def _build_b(trp):
    nc = bacc.Bacc("TRN2", target_bir_lowering=False, debug=False,
                   num_devices=N_CORES)

    x_h = nc.declare_dram_parameter("x_h", [128, KT, TLOC], f16,
                                    isOutput=False)
    wfc = nc.declare_dram_parameter("wfc", [128, KT, C], f16, isOutput=False)
    wproj = nc.declare_dram_parameter("wproj", [128, KT, C], f16,
                                      isOutput=False)
    xg = nc.declare_dram_parameter("xg", [128, KT, trp], f16, isOutput=False)
    w1 = nc.declare_dram_parameter("w1", [128, KT, C], f16, isOutput=False)
    w2 = nc.declare_dram_parameter("w2", [128, KT, C], f16, isOutput=False)
    o_ysh = nc.declare_dram_parameter("o_ysh", [C, TLOC], f16, isOutput=True)
    o_yr = nc.declare_dram_parameter("o_yr", [C, trp], f16, isOutput=True)

    with tile.TileContext(nc) as tc:
        with (
            tc.tile_pool(name="acts", bufs=1) as apool,
            tc.tile_pool(name="tbuf", bufs=2) as tpool,
            tc.tile_pool(name="ybuf", bufs=2) as ypool,
            tc.tile_pool(name="ps", bufs=4, space="PSUM") as pspool,
        ):
            # Per-k DMAs in consumption order; triggers split across the
            # sync and scalar HWDGE queues so issue latency stays off the
            # critical path (sync: x_h, w1, wproj, w2; scalar: wfc, xg).
            xh = apool.tile([128, KT, TLOC], f16, tag="xh")
            wfcsb = apool.tile([128, KT, C], f16, tag="wfcsb")
            for k in range(KT):
                nc.sync.dma_start(xh[:, k, :], x_h[:, k, :])
                nc.scalar.dma_start(wfcsb[:, k, :], wfc[:, k, :])
            w1sb = apool.tile([128, KT, C], f16, tag="w1sb")
            xgt = apool.tile([128, KT, trp], f16, tag="xgt")
            for k in range(KT):
                nc.sync.dma_start(w1sb[:, k, :], w1[:, k, :])
                nc.scalar.dma_start(xgt[:, k, :], xg[:, k, :])
            wpsb = apool.tile([128, KT, C], f16, tag="wpsb")
            w2sb = apool.tile([128, KT, C], f16, tag="w2sb")
            for k in range(KT):
                nc.sync.dma_start(wpsb[:, k, :], wproj[:, k, :])
                nc.sync.dma_start(w2sb[:, k, :], w2[:, k, :])

            hsq_s = apool.tile([128, KT, TLOC], f16, tag="hsq_s")
            hsq_r = apool.tile([128, KT, trp], f16, tag="hsq_r")
            # sL1 -> rL1 -> sL2 -> rL2: each layer's PSUM drain finishes
            # well before its consumer starts, so the PE never waits.
            _emit_layer1(nc, pspool, tpool, wfcsb, xh, hsq_s, TLOC, ramp=True)
            _emit_layer1(nc, pspool, tpool, w1sb, xgt, hsq_r, trp)
            _emit_layer2(nc, pspool, ypool, wpsb, hsq_s, o_ysh, TLOC)
            _emit_layer2(nc, pspool, ypool, w2sb, hsq_r, o_yr, trp)
    nc.compile()
    return nc


_NC_A = None
_NC_B = {}


def _get_nc_a():
    global _NC_A
    if _NC_A is None:
        _NC_A = _build_a()
    return _NC_A


def _get_nc_b(trp):
    if trp not in _NC_B:
        _NC_B[trp] = _build_b(trp)
    return _NC_B[trp]


def _run(nc, in_maps, label):
    if TRACE:
        import tempfile
        td = tempfile.mkdtemp(prefix=f"moe_{label}_")
        res = run_bass_kernel_spmd(nc, in_maps, list(range(N_CORES)),
                                   trace=True, tmpdir=td)
        kernel._exec_ns[label] = res.exec_time_ns
        kernel._trace_dirs[label] = td
    else:
        res = run_bass_kernel_spmd(nc, in_maps, list(range(N_CORES)))
    return res


def _ptiles(a):
    """[C, t] -> [128, KT, t] partition-major layout, contiguous."""
    return np.ascontiguousarray(
        a.reshape(KT, 128, a.shape[1]).transpose(1, 0, 2))


def kernel(x, w_fc_sh, w_proj_sh, w1, w2, router_w, balance_bias):
    kernel._exec_ns = {}
    kernel._trace_dirs = {}

    xf = np.ascontiguousarray(np.asarray(x, np.float32).reshape(N_TOK, C))
    rwT = np.ascontiguousarray(np.asarray(router_w, np.float32).T)
    wfc16 = _ptiles(np.asarray(w_fc_sh, np.float32).astype(np.float16))
    wproj16 = _ptiles(np.asarray(w_proj_sh, np.float32).astype(np.float16))
    w1_16 = [_ptiles(np.asarray(w1[e], np.float32).astype(np.float16))
             for e in range(E)]
    w2_16 = [_ptiles(np.asarray(w2[e], np.float32).astype(np.float16))
             for e in range(E)]
    bias = np.asarray(balance_bias, np.float64)

    # ---- launch A: router logits, data-parallel ----
    nc_a = _get_nc_a()
    xTs = [np.ascontiguousarray(xf[i * TLOC:(i + 1) * TLOC].T)
           for i in range(N_CORES)]
    res_a = _run(nc_a, [{"x_T": _ptiles(xTs[i]), "rwT": rwT}
                        for i in range(N_CORES)], "a")

    lg = np.concatenate([res_a.results[i]["o_lg"].T
                         for i in range(N_CORES)], axis=0)      # [N, E] fp32

    # ---- host dispatch: top-2 selection + per-expert gather ----
    scores = 1.0 / (1.0 + np.exp(-lg.astype(np.float64)))
    idx = np.argsort(-(scores + bias[None, :]), axis=-1, kind="stable")[:, :K]
    tw = np.take_along_axis(scores, idx, -1)
    tw = tw / (tw.sum(-1, keepdims=True) + 1e-20)
    comb = np.zeros((N_TOK, E))
    np.put_along_axis(comb, idx, tw, -1)

    tok_lists = [np.nonzero(comb[:, e])[0] for e in range(E)]
    trp = max(512, -(-max(len(t) for t in tok_lists) // 128) * 128)

    nc_b = _get_nc_b(trp)
    in_maps = []
    for e in range(E):
        te = tok_lists[e]
        xe = xf[te] * np.sqrt(comb[te, e]).astype(np.float32)[:, None]
        xgT = np.zeros((C, trp), np.float32)
        xgT[:, :len(te)] = xe.T
        in_maps.append({"x_h": _ptiles(xTs[e]).astype(np.float16),
                        "wfc": wfc16, "wproj": wproj16,
                        "xg": _ptiles(xgT).astype(np.float16),
                        "w1": w1_16[e], "w2": w2_16[e]})

    # ---- launch B: shared expert (own tokens) + routed expert e ----
    res_b = _run(nc_b, in_maps, "b")

    y = np.concatenate([res_b.results[i]["o_ysh"].T
                        for i in range(N_CORES)], axis=0).astype(np.float32)
    for e in range(E):
        te = tok_lists[e]
        y[te] += res_b.results[e]["o_yr"][:, :len(te)].T.astype(np.float32)

    kernel._comb = comb
    return y.reshape(B, T, C).astype(np.float32)
